# revision 1
# baseline (speedup 1.0000x reference)
"""NetVLAD-style vq_codebook kernel for 8 Trainium2 NeuronCores.

Reference computation (per full input):
  assn = BN(x @ clusters); softmax over 80 clusters, drop 16 ghosts
  vlad[b,d,k] = sum_n assn[b,n,k] x[b,n,d] - a_sum[b,k]*clusters2[d,k]
  intra-normalize over d, flatten, global L2 normalize -> (B, D*K)

Sharding: data-parallel over batch B (B/8 batches per core). BatchNorm
statistics (sum and sum-of-squares per cluster column) are all-reduced
across the 8 cores (2*80 floats). Everything else is local.

Implementation notes:
 - x is cast to fp16 on load (gpsimd cast-DMA), kept in natural layout
   (token-partition) for the vlad matmul, and transposed on-chip with the
   DMA XBAR transpose into d-partition layout for the assignment matmul.
 - PE matmuls: per token tile 4 accumulating (128x128fp16)@(128x80fp16)
   matmuls for cluster assignment; vlad: per token tile one
   (128x64)@(128x512) matmul accumulating vlad^T = (64k, 512d) per batch,
   plus an N=1 matmul against a ones column for a_sum.
 - BN stats via PE: ones-column stationary matmuls against assn and
   assn^2 accumulate per-column sums in PSUM.
 - softmax without max-subtraction (logits are exactly BN-normalized,
   |logit| <~ 6, exp is safe in fp32).
"""

import sys

for _p in ("/opt/trn_rl_repo", "/root/.axon_site/_ro/trn_rl_repo"):
    if _p not in sys.path:
        sys.path.insert(0, _p)

import numpy as np

import concourse.bacc as bacc
import concourse.mybir as mybir
import concourse.tile as tile
from concourse.bass_utils import run_bass_kernel_spmd

F32 = mybir.dt.float32
F16 = mybir.dt.float16
AX = mybir.AxisListType
OP = mybir.AluOpType
ACTF = mybir.ActivationFunctionType

N_CORES = 8
D = 512
KG = 80          # clusters + ghosts
K = 64           # real clusters
N_SEQ = 2048
TPB = N_SEQ // 128   # token tiles per batch = 16
BN_EPS = 1e-5
L2_EPS = 1e-12


def build(b_loc=4, n_cores=N_CORES, with_collective=True):
    """Build the per-core program. b_loc = batches per core."""
    nt = b_loc * TPB                # token tiles per core
    tok = nt * 128                  # tokens per core
    total_tok = tok * n_cores       # global token count for BN stats

    nc = bacc.Bacc("TRN2", target_bir_lowering=False, debug=False,
                   dynamic_dma_scratch_size=65536)

    x = nc.declare_dram_parameter("x", [tok, D], F32, isOutput=False)
    cl = nc.declare_dram_parameter("clusters", [D, KG], F32, isOutput=False)
    c2 = nc.declare_dram_parameter("clusters2", [D, K], F32, isOutput=False)
    gam = nc.declare_dram_parameter("bn_gamma", [1, KG], F32, isOutput=False)
    bet = nc.declare_dram_parameter("bn_beta", [1, KG], F32, isOutput=False)
    y = nc.declare_dram_parameter("y", [b_loc, D * K], F32, isOutput=True)

    ones_row_c = nc.inline_tensor(np.ones((1, 128), np.float32), name="c_ones_row")

    with tile.TileContext(nc) as tc:
        with (
            tc.tile_pool(name="persist", bufs=1) as persist,
            tc.tile_pool(name="work", bufs=4) as work,
            tc.tile_pool(name="dram", bufs=1, space="DRAM") as dram,
        ):
            # ---- persistent SBUF tensors ----
            xh = persist.tile([128, nt, D], F16, name="xh")
            ones16 = persist.tile([128, 1], F16, name="ones16")
            assn = persist.tile([128, nt, KG], F16, name="assn")
            asqP = persist.tile([128, nt, KG], F16, name="asqP")
            sm = persist.tile([128, nt, K], F16, name="sm")
            clh = persist.tile([128, 4, KG], F16, name="clh")
            c2n = persist.tile([128, 4, K], F32, name="c2n")
            ones_row = persist.tile([1, 128], F32, name="ones_row")
            gamma = persist.tile([1, KG], F32, name="gamma")
            beta = persist.tile([1, KG], F32, name="beta")
            ss = persist.tile([1, 2 * KG], F32, name="ss")
            stats_sb = persist.tile([1, 2 * KG], F32, name="stats_sb")
            stats_g = persist.tile([1, 2 * KG], F32, name="stats_g")
            bcB = persist.tile([128, 2 * KG], F16, name="bcB")

            stats_in = dram.tile([1, 2 * KG], F32, name="stats_in")
            stats_out = dram.tile([1, 2 * KG], F32, name="stats_out")

            # ---- phase 0: constants + x load/cast ----
            nc.sync.dma_start(ones_row[:], ones_row_c.ap()[:, :])
            nc.sync.dma_start(gamma[:], gam[:, :])
            nc.sync.dma_start(beta[:], bet[:, :])
            # clusters -> fp16 chunks (cast dma): chunk c partition p = row 128c+p
            nc.gpsimd.dma_start(
                clh[:], cl.ap().rearrange("(c p) k -> p c k", p=128))
            # clusters2 natural layout; PE-transposed to (64k, 512d) below
            nc.sync.dma_start(
                c2n[:], c2.ap().rearrange("(c p) k -> p c k", p=128))
            nc.vector.memset(ones16[:], 1.0)

            # x cast-DMA in groups of 8 token tiles (SWDGE casts
            # fp32->fp16 in the DMA engines; HBM read is the real cost)
            xr = x.ap().rearrange("(t p) d -> p t d", p=128)
            for g in range(nt // 8):
                nc.gpsimd.dma_start(
                    xh[:, 8 * g:8 * (g + 1), :], xr[:, 8 * g:8 * (g + 1), :])

            # ---- phases 0b-2: transposes, assignment matmul, BN stats ----
            with tc.tile_pool(name="ps1", bufs=5, space="PSUM") as ps1:
                # BN stats accumulate in their own banks, pipelined one
                # tile-group behind the assignment matmuls (safe: start=True
                # clears has_written per-bank only)
                pstat_s = ps1.tile([1, 4 * KG], F32, name="pstat_s",
                                   tag="st_s", bufs=1)
                pstat_q = ps1.tile([1, 4 * KG], F32, name="pstat_q",
                                   tag="st_q", bufs=1)
                ng = nt // 4

                def emit_stats(g):
                    nc.tensor.matmul(pstat_s[:], ones16[:],
                                     assn[:, 4 * g:4 * g + 4, :],
                                     start=(g == 0), stop=(g == ng - 1),
                                     skip_group_check=True)
                    nc.tensor.matmul(pstat_q[:], ones16[:],
                                     asqP[:, 4 * g:4 * g + 4, :],
                                     start=(g == 0), stop=(g == ng - 1),
                                     skip_group_check=True)

                for tg in range(nt // 8):
                    xhTg = work.tile([128, 32, 128], F16, name="xhTg",
                                     tag="xhT", bufs=4)
                    # batched XBAR transpose: (128, 8*512) -> (128, 32, 128)
                    # with logical row 128*e + p at [:, e, :]; e = 4*j + c,
                    # d = 128*c + p (chunk-major per tile), matching clh
                    nc.sync.dma_start(xhTg[:, :, :],
                                      xh[:, 8 * tg:8 * (tg + 1), :],
                                      transpose=True)
                    for j in range(8):
                        t = 8 * tg + j
                        p1 = ps1.tile([128, KG], F32, name="p1", tag="p1")
                        for c in range(4):
                            nc.tensor.matmul(
                                p1[:], xhTg[:, 4 * j + c, :], clh[:, c, :],
                                start=(c == 0), stop=(c == 3),
                                skip_group_check=True)
                        nc.vector.tensor_copy(assn[:, t, :], p1[:])
                        if t % 4 == 3:
                            nc.scalar.square(asqP[:, t - 3:t + 1, :],
                                             assn[:, t - 3:t + 1, :])
                    if tg >= 1:
                        emit_stats(2 * (tg - 1))
                        emit_stats(2 * (tg - 1) + 1)
                emit_stats(ng - 2)
                emit_stats(ng - 1)


                # ---- phase 2: all-reduce stats ----
                nc.vector.tensor_reduce(
                    stats_sb[:, :KG],
                    pstat_s[:].rearrange("p (t k) -> p k t", t=4),
                    axis=AX.X, op=OP.add)
                nc.vector.tensor_reduce(
                    stats_sb[:, KG:],
                    pstat_q[:].rearrange("p (t k) -> p k t", t=4),
                    axis=AX.X, op=OP.add)

            nc.sync.dma_start(stats_in[:], stats_sb[:])
            if with_collective:
                nc.gpsimd.collective_compute(
                    "AllReduce", OP.add,
                    replica_groups=[list(range(n_cores))],
                    ins=[stats_in.opt()], outs=[stats_out.opt()])
            else:
                nc.sync.dma_start(stats_out[:], stats_in[:])
            nc.sync.dma_start(stats_g[:], stats_out[:])

            t_mean = work.tile([1, KG], F32, name="t_mean", tag="sv", bufs=6)
            t_var = work.tile([1, KG], F32, name="t_var", tag="sv", bufs=6)
            t_sd = work.tile([1, KG], F32, name="t_sd", tag="sv", bufs=6)
            t_rs = work.tile([1, KG], F32, name="t_rs", tag="sv", bufs=6)
            t_ms = work.tile([1, KG], F32, name="t_ms", tag="sv", bufs=6)
            inv_n = 1.0 / float(total_tok)
            nc.vector.tensor_scalar_mul(t_mean[:], stats_g[:, :KG], inv_n)
            nc.vector.tensor_scalar_mul(t_var[:], stats_g[:, KG:], inv_n)
            nc.vector.tensor_tensor(t_ms[:], t_mean[:], t_mean[:], op=OP.mult)
            nc.vector.tensor_tensor(t_var[:], t_var[:], t_ms[:], op=OP.subtract)
            nc.vector.tensor_scalar_add(t_var[:], t_var[:], BN_EPS)
            nc.scalar.sqrt(t_sd[:], t_var[:])
            nc.vector.reciprocal(t_rs[:], t_sd[:])
            nc.vector.tensor_tensor(ss[:, :KG], t_rs[:], gamma[:], op=OP.mult)
            nc.vector.tensor_tensor(t_ms[:], t_mean[:], ss[:, :KG], op=OP.mult)
            nc.vector.tensor_tensor(ss[:, KG:], beta[:], t_ms[:], op=OP.subtract)

            # ---- phases 3-5: softmax (all batches first, one Exp LUT
            # load), vlad matmul with x stationary -> natural (d,k) layout,
            # then normalization (software-pipelined across batches) ----
            with (
                tc.tile_pool(name="ps2", bufs=2, space="PSUM") as ps2,
                tc.tile_pool(name="elem", bufs=4) as elem,
                tc.tile_pool(name="vpost", bufs=3) as vpost,
            ):
                pbc = ps2.tile([128, 2 * KG], F32, name="pbc", tag="bc2")
                nc.tensor.matmul(pbc[:], ones_row[:], ss[:], start=True,
                                 stop=True, skip_group_check=True)
                nc.vector.tensor_copy(bcB[:], pbc[:])
                scale_b = bcB[:, :KG].rearrange("p (a k) -> p a k", a=1)
                shift_b = bcB[:, KG:].rearrange("p (a k) -> p a k", a=1)

                for b in range(b_loc):
                    t0 = b * TPB
                    te = elem.tile([128, TPB, KG], F16, name="te", tag="te")
                    nc.vector.tensor_tensor(
                        te[:], assn[:, t0:t0 + TPB, :],
                        scale_b.to_broadcast([128, TPB, KG]), op=OP.mult)
                    nc.vector.tensor_tensor(
                        te[:], te[:], shift_b.to_broadcast([128, TPB, KG]),
                        op=OP.add)
                    nc.scalar.activation(te[:], te[:], ACTF.Exp)
                    denom = work.tile([128, TPB], F16, name="denom", tag="dn")
                    with nc.allow_low_precision("fp16 softmax denom"):
                        nc.vector.tensor_reduce(denom[:], te[:], axis=AX.X,
                                                op=OP.add)
                    recip = work.tile([128, TPB], F16, name="recip", tag="rc")
                    with nc.allow_low_precision("fp16 softmax recip"):
                        nc.vector.reciprocal(recip[:], denom[:])
                    nc.vector.tensor_tensor(
                        sm[:, t0:t0 + TPB, :], te[:, :, :K],
                        recip[:].rearrange("p (t a) -> p t a", a=1)
                        .to_broadcast([128, TPB, K]), op=OP.mult)

                state = {}

                def mm_stage(b):
                    t0 = b * TPB
                    pv2 = ps2.tile([128, 4 * K], F32, name="pv2", tag="pv")
                    pas = ps2.tile([1, 4 * K], F32, name="pas", tag="pas")
                    pv3 = pv2[:].rearrange("p (c k) -> p c k", c=4)
                    # NOTE: groups must be contiguous per PSUM bank region --
                    # start=True clears has_written for the whole bank, so
                    # interleaving c-groups drops earlier partial sums.
                    for c in range(4):
                        for i in range(TPB):
                            t = t0 + i
                            nc.tensor.matmul(
                                pv3[:, c, :],
                                xh[:, t, c * 128:(c + 1) * 128],
                                sm[:, t, :],
                                start=(i == 0), stop=(i == TPB - 1),
                                skip_group_check=True)
                    for g in range(TPB // 4):
                        nc.tensor.matmul(pas[:], ones16[:],
                                         sm[:, t0 + 4 * g:t0 + 4 * g + 4, :],
                                         start=(g == 0), stop=(g == TPB // 4 - 1),
                                         skip_group_check=True)
                    state[b] = (pv2, pas)

                def post_stage(b):
                    pv2, pas = state.pop(b)
                    pv3 = pv2[:].rearrange("p (c k) -> p c k", c=4)
                    pa_sb = work.tile([1, K], F32, name="pa_sb", tag="pas_sb")
                    nc.vector.tensor_reduce(
                        pa_sb[:], pas[:].rearrange("p (i k) -> p k i", i=4),
                        axis=AX.X, op=OP.add)
                    pamB = ps2.tile([128, K], F32, name="pamB", tag="bc2")
                    nc.tensor.matmul(pamB[:], ones_row[:], pa_sb[:],
                                     start=True, stop=True,
                                     skip_group_check=True)
                    # v = vlad - a_sum*clusters2 in natural (p, c, k) layout
                    av = vpost.tile([128, 4, K], F32, name="av", tag="av")
                    nc.vector.tensor_tensor(
                        av[:], c2n[:],
                        pamB[:].rearrange("p (a k) -> p a k", a=1)
                        .to_broadcast([128, 4, K]), op=OP.mult)
                    v = vpost.tile([128, 4, K], F32, name="v", tag="v")
                    nc.vector.tensor_tensor(v[:], pv3[:], av[:],
                                            op=OP.subtract)
                    # intra-norm over d (partitions x chunks) via PE
                    sq = vpost.tile([128, 4, K], F16, name="sq", tag="sq")
                    with nc.allow_low_precision("fp16 norm squares"):
                        nc.vector.tensor_tensor(sq[:], v[:], v[:], op=OP.mult)
                    pnrm = ps2.tile([1, 4 * K], F32, name="pnrm", tag="pnrm")
                    nc.tensor.matmul(pnrm[:], ones16[:], sq[:], start=True,
                                     stop=True, skip_group_check=True)
                    nrm2 = work.tile([1, K], F32, name="nrm2", tag="nr")
                    nc.vector.tensor_reduce(
                        nrm2[:], pnrm[:].rearrange("p (c k) -> p k c", c=4),
                        axis=AX.X, op=OP.add)
                    snorm = work.tile([1, K], F32, name="snorm", tag="nr")
                    nc.scalar.sqrt(snorm[:], nrm2[:])
                    nc.vector.tensor_scalar_max(snorm[:], snorm[:], L2_EPS)
                    rn = work.tile([1, K], F32, name="rn", tag="nr")
                    nc.vector.reciprocal(rn[:], snorm[:])
                    # global norm: g2 = sum_k (snorm*rn)^2
                    t1 = work.tile([1, K], F32, name="t1", tag="nr")
                    nc.vector.tensor_tensor(t1[:], snorm[:], rn[:], op=OP.mult)
                    nc.vector.tensor_tensor(t1[:], t1[:], t1[:], op=OP.mult)
                    g2 = work.tile([1, 1], F32, name="g2", tag="g1", bufs=6)
                    nc.vector.tensor_reduce(g2[:], t1[:], axis=AX.X, op=OP.add)
                    gs = work.tile([1, 1], F32, name="gs", tag="g1", bufs=6)
                    nc.scalar.sqrt(gs[:], g2[:])
                    nc.vector.tensor_scalar_max(gs[:], gs[:], L2_EPS)
                    gr = work.tile([1, 1], F32, name="gr", tag="g1", bufs=6)
                    nc.vector.reciprocal(gr[:], gs[:])
                    nc.vector.tensor_scalar(rn[:], rn[:], gr[:], None,
                                            op0=OP.mult)
                    prnB = ps2.tile([128, K], F32, name="prnB", tag="bc2")
                    nc.tensor.matmul(prnB[:], ones_row[:], rn[:], start=True,
                                     stop=True, skip_group_check=True)
                    vf = vpost.tile([128, 4, K], F32, name="vf", tag="vf")
                    nc.vector.tensor_tensor(
                        vf[:], v[:],
                        prnB[:].rearrange("p (a k) -> p a k", a=1)
                        .to_broadcast([128, 4, K]), op=OP.mult)
                    yb = y[b, :].rearrange("(c p k) -> p c k", p=128, k=K)
                    nc.sync.dma_start(yb[:, :, :], vf[:])

                for b in range(b_loc):
                    mm_stage(b)
                    if b >= 1:
                        post_stage(b - 1)
                post_stage(b_loc - 1)
    nc.compile()
    return nc


_CACHE = {}


def _get(b_loc, n_cores, with_collective):
    key = (b_loc, n_cores, with_collective)
    if key not in _CACHE:
        _CACHE[key] = build(b_loc, n_cores, with_collective)
    return _CACHE[key]


def make_in_maps(x, clusters, clusters2, bn_gamma, bn_beta, n_cores=N_CORES):
    B = x.shape[0]
    b_loc = B // n_cores
    shared = {
        "clusters": np.ascontiguousarray(clusters, np.float32),
        "clusters2": np.ascontiguousarray(
            np.asarray(clusters2).reshape(D, K), np.float32),
        "bn_gamma": np.ascontiguousarray(
            np.asarray(bn_gamma).reshape(1, KG), np.float32),
        "bn_beta": np.ascontiguousarray(
            np.asarray(bn_beta).reshape(1, KG), np.float32),
    }
    in_maps = []
    for i in range(n_cores):
        m = dict(shared)
        m["x"] = np.ascontiguousarray(
            np.asarray(x[i * b_loc:(i + 1) * b_loc]).reshape(
                b_loc * N_SEQ, D), np.float32)
        in_maps.append(m)
    return in_maps


def kernel(x, clusters, clusters2, bn_gamma, bn_beta):
    B, N, Dd = x.shape
    assert (N, Dd) == (N_SEQ, D) and B % N_CORES == 0
    b_loc = B // N_CORES
    nc = _get(b_loc, N_CORES, True)
    in_maps = make_in_maps(x, clusters, clusters2, bn_gamma, bn_beta)
    res = run_bass_kernel_spmd(nc, in_maps, core_ids=list(range(N_CORES)))
    out = np.concatenate([res.results[i]["y"] for i in range(N_CORES)], axis=0)
    return out



# revision 37
# speedup vs baseline: 1.2758x; 1.2758x over previous
"""NetVLAD-style vq_codebook kernel for 8 Trainium2 NeuronCores.

Reference computation (per full input):
  assn = BN(x @ clusters); softmax over 80 clusters, drop 16 ghosts
  vlad[b,d,k] = sum_n assn[b,n,k] x[b,n,d] - a_sum[b,k]*clusters2[d,k]
  intra-normalize over d, flatten, global L2 normalize -> (B, D*K)

Sharding: data-parallel over batch B (B/8 batches per core). BatchNorm
statistics (sum and sum-of-squares per cluster column, 2*80 floats) are
all-reduced across the 8 cores. Everything else is local.

Schedule (per core):
  Phase A (DMA-paced, ~52us floor): 16 groups of 4 token tiles; per group
    a cast-DMA load of x (fp32->fp16), an XBAR transpose to d-partition
    layout, 16 assignment matmuls, PSUM->SBUF copy of the logits, an f16
    square, and two accumulating BN-stats ones-matmuls (lagged 2 groups so
    the PE never stalls on the DVE square).
  Barrier: stats DRAM round-trip (stands in for / carries the AllReduce),
    BN affine chain, broadcast of scale|shift to all partitions.
  Phase B: per batch softmax (DVE muls + one Exp + row-sum + recip; the
    renormalize multiply is split DVE/Act to balance engines), then the
    vlad matmul with x stationary, a_sum ones-matmuls, and the
    a_sum*clusters2 correction on GPSIMD.
  Tail (batched over the 4 local batches): squares, intra-norm ones-
    matmuls, rsqrt chain. The reference's global L2 norm over the
    flattened, intra-normalized vlad is exactly sqrt(K)=8 (every column
    has unit norm), so it is folded in as a constant 1/8 scale.
"""

import sys

for _p in ("/opt/trn_rl_repo", "/root/.axon_site/_ro/trn_rl_repo"):
    if _p not in sys.path:
        sys.path.insert(0, _p)

import numpy as np

import concourse.bacc as bacc
import concourse.mybir as mybir
import concourse.tile as tile
from concourse.bass_utils import run_bass_kernel_spmd

F32 = mybir.dt.float32
F16 = mybir.dt.float16
AX = mybir.AxisListType
OP = mybir.AluOpType
ACTF = mybir.ActivationFunctionType

N_CORES = 8
D = 512
KG = 80          # clusters + ghosts
K = 64           # real clusters
N_SEQ = 2048
TPB = N_SEQ // 128   # token tiles per batch = 16
BN_EPS = 1e-5
L2_EPS = 1e-12


def build(b_loc=4, n_cores=N_CORES, with_collective=True, debug_taps=False):
    """Build the per-core program. b_loc = batches per core."""
    nt = b_loc * TPB                # token tiles per core = 64
    tok = nt * 128                  # tokens per core
    total_tok = tok * n_cores       # global token count for BN stats
    ngrp = nt // 4                  # 4-tile DMA/compute groups

    nc = bacc.Bacc("TRN2", target_bir_lowering=False, debug=False,
                   dynamic_dma_scratch_size=32768)

    x = nc.declare_dram_parameter("x", [tok, D], F32, isOutput=False)
    cl = nc.declare_dram_parameter("clusters", [D, KG], F32, isOutput=False)
    c2 = nc.declare_dram_parameter("clusters2", [D, K], F32, isOutput=False)
    gam = nc.declare_dram_parameter("bn_gamma", [1, KG], F32, isOutput=False)
    bet = nc.declare_dram_parameter("bn_beta", [1, KG], F32, isOutput=False)
    y = nc.declare_dram_parameter("y", [b_loc, D * K], F32, isOutput=True)
    if debug_taps:
        nt_ = b_loc * TPB
        dbg_aq = nc.declare_dram_parameter(
            "dbg_aq", [128, nt_ * 2 * KG], F32, isOutput=True)
        dbg_sm = nc.declare_dram_parameter(
            "dbg_sm", [128, nt_ * K], F32, isOutput=True)
        dbg_st = nc.declare_dram_parameter(
            "dbg_st", [1, 2 * KG], F32, isOutput=True)
        dbg_vv = nc.declare_dram_parameter(
            "dbg_vv", [128, b_loc * 4 * K], F32, isOutput=True)
        dbg_xh = nc.declare_dram_parameter(
            "dbg_xh", [128, 2 * TPB * D], F32, isOutput=True)
        dbg_cl = nc.declare_dram_parameter(
            "dbg_cl", [128, 4 * KG], F32, isOutput=True)

    ones_row_c = nc.inline_tensor(np.ones((1, 128), np.float32), name="c_ones_row")

    with tile.TileContext(nc) as tc:
        with (
            tc.tile_pool(name="persist", bufs=1) as persist,
            tc.tile_pool(name="work", bufs=4) as work,
            tc.tile_pool(name="dram", bufs=1, space="DRAM") as dram,
        ):
            # ---- persistent SBUF tensors ----
            # x lives in two tiles of two batches each; each tile has
            # exactly one DMA writer (a single big tile makes the
            # dependency tracker collapse subtile intervals and serialize
            # loads behind transposes, and >2 SWDGE loads in flight pick
            # up scheduler-inserted waits on unrelated transposes)
            xht = [persist.tile([128, 2, TPB, D], F16, name=f"xh{h}")
                   for h in range(b_loc // 2)]
            # interleaved logits|squares so one stats matmul covers both
            aq = persist.tile([128, nt, 2, KG], F16, name="aq")
            sm = persist.tile([128, nt, K], F16, name="sm")
            clh = persist.tile([128, 4, KG], F16, name="clh")
            c2n = persist.tile([128, 4, K], F32, name="c2n")
            ones16 = persist.tile([128, 1], F16, name="ones16")
            ones32 = persist.tile([128, 1], F32, name="ones32")
            epsc = persist.tile([1, 1], F32, name="epsc")
            ones_row = persist.tile([1, 128], F32, name="ones_row")
            gamma = persist.tile([1, KG], F32, name="gamma")
            beta = persist.tile([1, KG], F32, name="beta")
            stats_sb = persist.tile([1, 2 * KG], F32, name="stats_sb")
            stats_g = persist.tile([1, 2 * KG], F32, name="stats_g")
            ss = persist.tile([1, 2 * KG], F32, name="ss")
            bcB = persist.tile([128, 2, KG], F16, name="bcB")
            vv = persist.tile([128, b_loc, 4, K], F32, name="vv")
            vf = persist.tile([128, b_loc, 4, K], F32, name="vf")
            pa_sb = persist.tile([1, b_loc, K], F32, name="pa_sb")
            dummy = persist.tile([1, 1], F32, name="dummy")

            stats_in = dram.tile([1, 2 * KG], F32, name="stats_in")
            stats_out = dram.tile([1, 2 * KG], F32, name="stats_out")


            # ---- phase A: load + transpose + assignment + BN stats ----
            # Token permutation: xh[b][p, t] = x[b, 16*p + t] with
            # partition p holding 16 consecutive tokens of each batch
            # ("(b p t) d" order). Tokens are exchangeable within a batch
            # (BN stats, softmax, vlad, a_sum are all order-invariant);
            # this makes every partition's HBM read contiguous, so each
            # batch loads with ~128 DMA descriptors in one instruction.
            # DMA instructions carry a fixed ~2-3us serial launch cost in
            # the scheduler, so phase A uses as few as possible: 4 batch
            # loads + 4 batch XBAR transposes. ALL loads are emitted
            # before any transpose (a load emitted after a transpose picks
            # up a false WAR edge and serializes).
            xr = x.ap().rearrange("(b p t) d -> p b (t d)", p=128, t=TPB)
            for h in range(b_loc // 2):
                nc.gpsimd.dma_start(
                    xht[h][:].rearrange("p b t d -> p b (t d)"),
                    xr[:, 2 * h:2 * h + 2, :])
            # constants after the x loads (so the first load's descriptor
            # generation isn't queued behind them); preload the Sqrt act
            # table while the Act engine is idle
            nc.sync.dma_start(ones_row[:], ones_row_c.ap()[:, :])
            nc.sync.dma_start(gamma[:], gam[:, :])
            nc.sync.dma_start(beta[:], bet[:, :])
            nc.gpsimd.dma_start(
                clh[:], cl.ap().rearrange("(c p) k -> p c k", p=128))
            nc.sync.dma_start(
                c2n[:], c2.ap().rearrange("(c p) k -> p c k", p=128))
            nc.vector.memset(ones16[:], 1.0)
            nc.vector.memset(ones32[:], 1.0)
            nc.vector.memset(epsc[:], BN_EPS)
            nc.vector.memset(dummy[:], 1.0)
            nc.scalar.sqrt(dummy[:], dummy[:])
            with tc.tile_pool(name="psA", bufs=1, space="PSUM") as psA:
                pstat = psA.tile([1, 2, 2, KG], F32, name="pstat",
                                 tag="pstat", bufs=1)

                def emit_stats(q):
                    # accumulating [1, 2*2*KG=320] ones-matmuls per tile pair
                    for h in range(2):
                        s0 = 4 * q + 2 * h
                        nc.tensor.matmul(
                            pstat[:], ones16[:], aq[:, s0:s0 + 2, :, :],
                            start=(s0 == 0), stop=(s0 == nt - 2),
                            skip_group_check=True)

                # 8-tile (4096-column) slabs. On hardware the XBAR
                # transpose's completion semaphore fires when the ucode is
                # dispatched, not when the data lands, so a consumer keyed
                # on it races the transpose. All transposes go on ONE DGE
                # queue (FIFO): a transpose's dispatch therefore implies
                # the previous transpose's data is fully written. Each
                # slab's assignment matmuls are fenced by a guard matmul
                # that reads slot 0 of the NEXT slab's transpose; the last
                # slab is fenced by a small same-queue self-copy instead.
                slabs = [(t0s // TPB, t0s, 8) for t0s in range(0, nt, 8)]
                xhTgs = []

                def emit_slab(sj):
                    bj, t0j, nj = slabs[sj]
                    xhT = xhTgs[sj]
                    for q in range(nj // 4):
                        p1 = psA.tile([128, 4, KG], F32, name="p1",
                                      tag="p1", bufs=3)
                        for j in range(4):
                            for c in range(4):
                                nc.tensor.matmul(
                                    p1[:, j, :],
                                    xhT[:, 16 * q + 4 * j + c, :],
                                    clh[:, c, :], start=(c == 0),
                                    stop=(c == 3), skip_group_check=True)
                        # stats lag a chunk behind so the PE never waits
                        # on the DVE square of the current chunk
                        qq = t0j // 4 + q
                        if qq >= 1:
                            emit_stats(qq - 1)
                        h0 = 4 * qq
                        nc.vector.tensor_copy(aq[:, h0:h0 + 4, 0, :],
                                              p1[:])
                        with nc.allow_low_precision("fp16 logit squares"):
                            nc.vector.tensor_tensor(
                                aq[:, h0:h0 + 4, 1, :],
                                aq[:, h0:h0 + 4, 0, :],
                                aq[:, h0:h0 + 4, 0, :], op=OP.mult)

                for si, (b, t0s, ntl) in enumerate(slabs):
                    xhTg = work.tile([128, 32, 128], F16, name="xhTg",
                                     tag="xhT", bufs=3)
                    # XBAR transpose: (128, ntl*512) -> (128, 4*ntl, 128)
                    # with logical row 128*e + p at [:, e, :]; e = 4*j + c,
                    # d = 128*c + p (chunk-major per tile), matching clh
                    toff = t0s - TPB * b
                    nc.sync.dma_start(
                        xhTg[:, :4 * ntl, :],
                        xht[b // 2][:, b % 2, toff:toff + ntl, :],
                        transpose=True)
                    xhTgs.append(xhTg)
                    if si >= 1:
                        pguard = psA.tile([128, 1], F32, name="pguard",
                                          tag="guard", bufs=2)
                        nc.tensor.matmul(pguard[:], xhTg[:, 0, :],
                                         ones16[:], start=True, stop=True,
                                         skip_group_check=True)
                        emit_slab(si - 1)
                # last slab: same-queue self-copy guard
                nc.sync.dma_start(xhTgs[-1][:, 31, :], xhTgs[-1][:, 31, :])
                pguard = psA.tile([128, 1], F32, name="pguard",
                                  tag="guard", bufs=2)
                nc.tensor.matmul(pguard[:], xhTgs[-1][:, 31, :], ones16[:],
                                 start=True, stop=True,
                                 skip_group_check=True)
                emit_slab(len(slabs) - 1)
                emit_stats(nt // 4 - 1)

                # fold the tile-pair axis: [1,(i,s,k)] -> [1,(s,k)]
                nc.vector.tensor_reduce(
                    stats_sb[:].rearrange("p (s k) -> p s k", s=2),
                    pstat[:].rearrange("p i s k -> p s k i"),
                    axis=AX.X, op=OP.add)

            # ---- all-reduce stats ----
            nc.sync.dma_start(stats_in[:], stats_sb[:])
            if with_collective:
                nc.gpsimd.collective_compute(
                    "AllReduce", OP.add,
                    replica_groups=[list(range(n_cores))],
                    ins=[stats_in.opt()], outs=[stats_out.opt()])
            else:
                nc.sync.dma_start(stats_out[:], stats_in[:])
            nc.sync.dma_start(stats_g[:], stats_out[:])

            # ---- BN affine: scale = gamma*rsqrt(var+eps); shift = beta-mean*scale
            mq = work.tile([1, 2 * KG], F32, name="mq", tag="sv", bufs=6)
            msq = work.tile([1, KG], F32, name="msq", tag="sv", bufs=6)
            var = work.tile([1, KG], F32, name="var", tag="sv", bufs=6)
            sd = work.tile([1, KG], F32, name="sd", tag="sv", bufs=6)
            rsd = work.tile([1, KG], F32, name="rsd", tag="sv", bufs=6)
            t1 = work.tile([1, KG], F32, name="t1", tag="sv", bufs=6)
            inv_n = 1.0 / float(total_tok)
            nc.vector.tensor_scalar_mul(mq[:], stats_g[:], inv_n)
            nc.vector.tensor_tensor(msq[:], mq[:, :KG], mq[:, :KG], op=OP.mult)
            nc.vector.tensor_tensor(var[:], mq[:, KG:], msq[:], op=OP.subtract)
            nc.scalar.activation(sd[:], var[:], ACTF.Sqrt, bias=epsc[:])
            nc.vector.reciprocal(rsd[:], sd[:])
            nc.vector.tensor_tensor(ss[:, :KG], rsd[:], gamma[:], op=OP.mult)
            nc.vector.tensor_tensor(t1[:], mq[:, :KG], ss[:, :KG], op=OP.mult)
            nc.vector.tensor_tensor(ss[:, KG:], beta[:], t1[:], op=OP.subtract)

            # ---- phase B: softmax + vlad + normalization ----
            with tc.tile_pool(name="psB", bufs=1, space="PSUM") as psB:
                pbc = psB.tile([128, 2 * KG], F32, name="pbc",
                               tag="misc", bufs=2)
                nc.tensor.matmul(pbc[:], ones_row[:], ss[:], start=True,
                                 stop=True, skip_group_check=True)
                nc.vector.tensor_copy(bcB[:].rearrange("p s k -> p (s k)"),
                                      pbc[:])
                scale_b = bcB[:, 0:1, :]
                shift_b = bcB[:, 1:2, :]

                te_tiles = {}

                def s1a(b):
                    # te = assn*scale + shift; exp. For b0 the ops are
                    # emitted in two half-batch pieces so the first
                    # exp/denominator can start ~1.5us earlier (pipeline
                    # fill); later batches overlap and use one piece.
                    t0 = b * TPB
                    te = work.tile([128, TPB, KG], F16, name="te",
                                   tag="te", bufs=2)
                    pieces = ((0, TPB // 2), (TPB // 2, TPB)) if b == 0 \
                        else ((0, TPB),)
                    for (ta, tb) in pieces:
                        n = tb - ta
                        nc.vector.tensor_tensor(
                            te[:, ta:tb], aq[:, t0 + ta:t0 + tb, 0, :],
                            scale_b.to_broadcast([128, n, KG]), op=OP.mult)
                        nc.vector.tensor_tensor(
                            te[:, ta:tb], te[:, ta:tb],
                            shift_b.to_broadcast([128, n, KG]), op=OP.add)
                        nc.scalar.activation(te[:, ta:tb], te[:, ta:tb],
                                             ACTF.Exp)
                    te_tiles[b] = te

                def s1b(b):
                    # denominators + renormalize; renorm split DVE/Act
                    # (DVE produces the first half, which the vlad matmul
                    # consumes first). b0 is emitted per half so its first
                    # sm tiles are ready ~1.5us sooner (pipeline fill).
                    t0 = b * TPB
                    te = te_tiles.pop(b)
                    half = TPB // 2
                    denom = work.tile([128, TPB], F16, name="denom",
                                      tag="dn", bufs=2)
                    # fp32: the Act Copy's scale AP must be FP32
                    recip = work.tile([128, TPB], F32, name="recip",
                                      tag="rc", bufs=2)
                    pieces = ((0, half), (half, TPB)) if b == 0 \
                        else ((0, TPB),)
                    for (ta, tb) in pieces:
                        with nc.allow_low_precision("fp16 softmax denom"):
                            nc.vector.tensor_reduce(
                                denom[:, ta:tb], te[:, ta:tb], axis=AX.X,
                                op=OP.add)
                        nc.vector.reciprocal(recip[:, ta:tb],
                                              denom[:, ta:tb])
                        mid = min(tb, max(ta, half))
                        if mid > ta:
                            nc.vector.tensor_tensor(
                                sm[:, t0 + ta:t0 + mid, :],
                                te[:, ta:mid, :K],
                                recip[:, ta:mid]
                                .rearrange("p (t a) -> p t a", a=1)
                                .to_broadcast([128, mid - ta, K]),
                                op=OP.mult)
                        if tb > mid:
                            with nc.allow_low_precision("fp16 softmax"):
                                nc.gpsimd.tensor_tensor(
                                    sm[:, t0 + mid:t0 + tb, :],
                                    te[:, mid:tb, :K],
                                    recip[:, mid:tb]
                                    .rearrange("p (t a) -> p t a", a=1)
                                    .to_broadcast([128, tb - mid, K]),
                                    op=OP.mult)

                def s2(b):
                    # vlad matmul with x stationary; the a_sum ones-matmuls
                    # go after the first chunk pass (all sm tiles consumed
                    # by then) so s3's pam/av overlap the remaining chunks
                    t0 = b * TPB
                    pas = psB.tile([1, K], F32, name="pas",
                                   tag="pas", bufs=2)
                    pv = psB.tile([128, 4, K], F32, name="pv",
                                  tag="pv", bufs=3)
                    # NOTE: groups must be contiguous per PSUM bank region --
                    # start=True clears has_written for the whole bank
                    for c in range(4):
                        for i in range(TPB):
                            t = t0 + i
                            nc.tensor.matmul(
                                pv[:, c, :],
                                xht[b // 2][:, b % 2, i,
                                            c * 128:(c + 1) * 128],
                                sm[:, t, :],
                                start=(i == 0), stop=(i == TPB - 1),
                                skip_group_check=True)
                        if c == 0:
                            for u in range(TPB):
                                nc.tensor.matmul(
                                    pas[:], ones16[:], sm[:, t0 + u, :],
                                    start=(u == 0), stop=(u == TPB - 1),
                                    skip_group_check=True)
                    return pv, pas

                def s3(b, pv, pas):
                    # pa_sb copy emitted here (one batch late) so the DVE
                    # queue never blocks on the PE's pas accumulation
                    nc.vector.tensor_copy(pa_sb[:, b, :], pas[:])
                    pam = psB.tile([128, K], F32, name="pam",
                                   tag="pam", bufs=1)
                    nc.tensor.matmul(pam[:], ones_row[:], pa_sb[:, b, :],
                                     start=True, stop=True,
                                     skip_group_check=True)
                    # GPSIMD cannot read PSUM on hardware: copy the pam
                    # broadcast to SBUF (DVE), compute av on GPSIMD, and do
                    # the pv subtraction on DVE
                    pam_sb = work.tile([128, K], F32, name="pam_sb",
                                       tag="pams", bufs=2)
                    nc.vector.tensor_copy(pam_sb[:], pam[:])
                    av = work.tile([128, 4, K], F32, name="av",
                                   tag="av", bufs=2)
                    nc.gpsimd.tensor_tensor(
                        av[:], c2n[:],
                        pam_sb[:].rearrange("p (a k) -> p a k", a=1)
                        .to_broadcast([128, 4, K]), op=OP.mult)
                    nc.vector.tensor_tensor(vv[:, b], pv[:], av[:],
                                            op=OP.subtract)

                yb = y.ap().rearrange("b (c p k) -> p b c k", p=128, k=K)

                def tail_one(b):
                    # normalization for batch b: intra-norm 1/||v||; the
                    # global norm of the flattened intra-normalized vlad is
                    # exactly sqrt(K)=8 -> fold 1/8. n2 = sum_d v*v comes
                    # from sq-matmuls: lhsT=v chunk gives columns v_k.v_k
                    # on... instead: accumulate ones32^T @ (v*v) is not a
                    # matmul; use v as both operands per chunk: out[k,k']
                    # too big -- so keep elementwise squares but in f32 on
                    # the PE via 4 accumulating matmuls over chunks of the
                    # f16 sq tile is replaced by: square on DVE only for
                    # this batch is skipped -- use matmul(ones32, vv) which
                    # sums v (not v^2); not valid. Keep the f16 square but
                    # write n2 with 4 accumulating ones-matmuls (no DVE
                    # reduce).
                    sq = work.tile([128, 4, K], F16, name="sq",
                                   tag="sq", bufs=2)
                    with nc.allow_low_precision("fp16 norm squares"):
                        nc.vector.tensor_tensor(
                            sq[:], vv[:, b], vv[:, b], op=OP.mult)
                    pnrm = psB.tile([1, 4, K], F32, name="pnrm",
                                    tag="misc", bufs=2)
                    nc.tensor.matmul(pnrm[:], ones16[:], sq[:], start=True,
                                     stop=True, skip_group_check=True)
                    n2 = work.tile([1, K], F32, name="n2", tag="n2",
                                   bufs=6)
                    nc.vector.tensor_reduce(
                        n2[:], pnrm[:].rearrange("p c k -> p k c"),
                        axis=AX.X, op=OP.add)
                    snorm = work.tile([1, K], F32, name="snorm",
                                      tag="n2", bufs=6)
                    nc.scalar.activation(snorm[:], n2[:], ACTF.Sqrt)
                    nc.vector.tensor_scalar(snorm[:], snorm[:], L2_EPS, 8.0,
                                            op0=OP.max, op1=OP.mult)
                    rn = work.tile([1, K], F32, name="rn", tag="n2",
                                   bufs=6)
                    nc.vector.reciprocal(rn[:], snorm[:])
                    prnB = psB.tile([128, K], F32, name="prnB",
                                    tag="misc", bufs=2)
                    nc.tensor.matmul(prnB[:], ones_row[:], rn[:],
                                     start=True, stop=True,
                                     skip_group_check=True)
                    nc.vector.tensor_tensor(
                        vf[:, b], vv[:, b],
                        prnB[:].rearrange("p (a k) -> p a k", a=1)
                        .to_broadcast([128, 4, K]), op=OP.mult)
                    dma_eng = nc.sync if b % 2 == 0 else nc.scalar
                    dma_eng.dma_start(yb[:, b], vf[:, b])

                # software pipeline: s1a one batch ahead; s3 lags one batch
                # so the PE's pam matmul never waits on the DVE reduce;
                # the first tail half (b0,b1) overlaps the b3 vlad work
                s1a(0)
                pvs = {}
                for b in range(b_loc):
                    if b + 1 < b_loc:
                        s1a(b + 1)
                    s1b(b)
                    pvs[b] = s2(b)
                    if b >= 1:
                        s3(b - 1, *pvs.pop(b - 1))
                        tail_one(b - 1)
                s3(b_loc - 1, *pvs.pop(b_loc - 1))
                tail_one(b_loc - 1)
                if debug_taps:
                    nc.gpsimd.dma_start(
                        dbg_aq.ap().rearrange("p (t s k) -> p t s k",
                                              t=nt, s=2), aq[:])
                    nc.gpsimd.dma_start(
                        dbg_sm.ap().rearrange("p (t k) -> p t k", t=nt),
                        sm[:])
                    nc.sync.dma_start(dbg_st.ap()[:, :], stats_g[:])
                    nc.sync.dma_start(
                        dbg_vv.ap().rearrange("p (b c k) -> p b c k",
                                              b=b_loc, c=4), vv[:])
                    nc.gpsimd.dma_start(
                        dbg_xh.ap().rearrange("p (b t d) -> p b t d",
                                              b=2, t=TPB), xht[0][:])
                    nc.gpsimd.dma_start(
                        dbg_cl.ap().rearrange("p (c k) -> p c k", c=4),
                        clh[:])
    nc.compile()
    return nc


_CACHE = {}


def _get(b_loc, n_cores, with_collective):
    key = (b_loc, n_cores, with_collective)
    if key not in _CACHE:
        _CACHE[key] = build(b_loc, n_cores, with_collective)
    return _CACHE[key]


def make_in_maps(x, clusters, clusters2, bn_gamma, bn_beta, n_cores=N_CORES):
    B = x.shape[0]
    b_loc = B // n_cores
    shared = {
        "clusters": np.ascontiguousarray(clusters, np.float32),
        "clusters2": np.ascontiguousarray(
            np.asarray(clusters2).reshape(D, K), np.float32),
        "bn_gamma": np.ascontiguousarray(
            np.asarray(bn_gamma).reshape(1, KG), np.float32),
        "bn_beta": np.ascontiguousarray(
            np.asarray(bn_beta).reshape(1, KG), np.float32),
    }
    in_maps = []
    for i in range(n_cores):
        m = dict(shared)
        m["x"] = np.ascontiguousarray(
            np.asarray(x[i * b_loc:(i + 1) * b_loc]).reshape(
                b_loc * N_SEQ, D), np.float32)
        in_maps.append(m)
    return in_maps


def kernel(x, clusters, clusters2, bn_gamma, bn_beta):
    B, N, Dd = x.shape
    assert (N, Dd) == (N_SEQ, D) and B % N_CORES == 0
    b_loc = B // N_CORES
    nc = _get(b_loc, N_CORES, True)
    in_maps = make_in_maps(x, clusters, clusters2, bn_gamma, bn_beta)
    res = run_bass_kernel_spmd(nc, in_maps, core_ids=list(range(N_CORES)))
    out = np.concatenate([res.results[i]["y"] for i in range(N_CORES)], axis=0)
    return out


# revision 41
# speedup vs baseline: 1.3421x; 1.0519x over previous
"""NetVLAD-style vq_codebook kernel for 8 Trainium2 NeuronCores.

Reference computation (per full input):
  assn = BN(x @ clusters); softmax over 80 clusters, drop 16 ghosts
  vlad[b,d,k] = sum_n assn[b,n,k] x[b,n,d] - a_sum[b,k]*clusters2[d,k]
  intra-normalize over d, flatten, global L2 normalize -> (B, D*K)

Sharding: data-parallel over batch B (B/8 batches per core). BatchNorm
statistics (sum and sum-of-squares per cluster column, 2*80 floats) are
all-reduced across the 8 cores. Everything else is local.

Schedule (per core):
  Phase A (DMA-paced, ~52us floor): 16 groups of 4 token tiles; per group
    a cast-DMA load of x (fp32->fp16), an XBAR transpose to d-partition
    layout, 16 assignment matmuls, PSUM->SBUF copy of the logits, an f16
    square, and two accumulating BN-stats ones-matmuls (lagged 2 groups so
    the PE never stalls on the DVE square).
  Barrier: stats DRAM round-trip (stands in for / carries the AllReduce),
    BN affine chain, broadcast of scale|shift to all partitions.
  Phase B: per batch softmax (DVE muls + one Exp + row-sum + recip; the
    renormalize multiply is split DVE/Act to balance engines), then the
    vlad matmul with x stationary, a_sum ones-matmuls, and the
    a_sum*clusters2 correction on GPSIMD.
  Tail (batched over the 4 local batches): squares, intra-norm ones-
    matmuls, rsqrt chain. The reference's global L2 norm over the
    flattened, intra-normalized vlad is exactly sqrt(K)=8 (every column
    has unit norm), so it is folded in as a constant 1/8 scale.
"""

import sys

for _p in ("/opt/trn_rl_repo", "/root/.axon_site/_ro/trn_rl_repo"):
    if _p not in sys.path:
        sys.path.insert(0, _p)

import numpy as np

import concourse.bacc as bacc
import concourse.mybir as mybir
import concourse.tile as tile
from concourse.bass_utils import run_bass_kernel_spmd

F32 = mybir.dt.float32
F16 = mybir.dt.float16
AX = mybir.AxisListType
OP = mybir.AluOpType
ACTF = mybir.ActivationFunctionType

N_CORES = 8
D = 512
KG = 80          # clusters + ghosts
K = 64           # real clusters
N_SEQ = 2048
TPB = N_SEQ // 128   # token tiles per batch = 16
BN_EPS = 1e-5
L2_EPS = 1e-12


def build(b_loc=4, n_cores=N_CORES, with_collective=True, debug_taps=False):
    """Build the per-core program. b_loc = batches per core."""
    nt = b_loc * TPB                # token tiles per core = 64
    tok = nt * 128                  # tokens per core
    total_tok = tok * n_cores       # global token count for BN stats
    ngrp = nt // 4                  # 4-tile DMA/compute groups

    nc = bacc.Bacc("TRN2", target_bir_lowering=False, debug=False,
                   dynamic_dma_scratch_size=32768)

    x = nc.declare_dram_parameter("x", [tok, D], F32, isOutput=False)
    cl = nc.declare_dram_parameter("clusters", [D, KG], F32, isOutput=False)
    c2 = nc.declare_dram_parameter("clusters2", [D, K], F32, isOutput=False)
    gam = nc.declare_dram_parameter("bn_gamma", [1, KG], F32, isOutput=False)
    bet = nc.declare_dram_parameter("bn_beta", [1, KG], F32, isOutput=False)
    y = nc.declare_dram_parameter("y", [b_loc, D * K], F32, isOutput=True)
    if debug_taps:
        nt_ = b_loc * TPB
        dbg_aq = nc.declare_dram_parameter(
            "dbg_aq", [128, nt_ * 2 * KG], F32, isOutput=True)
        dbg_sm = nc.declare_dram_parameter(
            "dbg_sm", [128, nt_ * K], F32, isOutput=True)
        dbg_st = nc.declare_dram_parameter(
            "dbg_st", [1, 2 * KG], F32, isOutput=True)
        dbg_vv = nc.declare_dram_parameter(
            "dbg_vv", [128, b_loc * 4 * K], F32, isOutput=True)
        dbg_xh = nc.declare_dram_parameter(
            "dbg_xh", [128, 2 * TPB * D], F32, isOutput=True)
        dbg_cl = nc.declare_dram_parameter(
            "dbg_cl", [128, 4 * KG], F32, isOutput=True)

    ones_row_c = nc.inline_tensor(np.ones((1, 128), np.float32), name="c_ones_row")

    with tile.TileContext(nc) as tc:
        with (
            tc.tile_pool(name="persist", bufs=1) as persist,
            tc.tile_pool(name="work", bufs=4) as work,
            tc.tile_pool(name="dram", bufs=1, space="DRAM") as dram,
        ):
            # ---- persistent SBUF tensors ----
            # x lives in two tiles of two batches each; each tile has
            # exactly one DMA writer (a single big tile makes the
            # dependency tracker collapse subtile intervals and serialize
            # loads behind transposes, and >2 SWDGE loads in flight pick
            # up scheduler-inserted waits on unrelated transposes)
            xht = [persist.tile([128, 2, TPB, D], F16, name=f"xh{h}")
                   for h in range(b_loc // 2)]
            # interleaved logits|squares so one stats matmul covers both
            aq = persist.tile([128, nt, 2, KG], F16, name="aq")
            sm = persist.tile([128, nt, K], F16, name="sm")
            clh = persist.tile([128, 4, KG], F16, name="clh")
            c2n = persist.tile([128, 4, K], F32, name="c2n")
            ones16 = persist.tile([128, 1], F16, name="ones16")
            ones32 = persist.tile([128, 1], F32, name="ones32")
            epsc = persist.tile([1, 1], F32, name="epsc")
            ones_row = persist.tile([1, 128], F32, name="ones_row")
            gamma = persist.tile([1, KG], F32, name="gamma")
            beta = persist.tile([1, KG], F32, name="beta")
            stats_sb = persist.tile([1, 2 * KG], F32, name="stats_sb")
            stats_g = persist.tile([1, 2 * KG], F32, name="stats_g")
            ss = persist.tile([1, 2 * KG], F32, name="ss")
            bcB = persist.tile([128, 2, KG], F16, name="bcB")
            vv = persist.tile([128, b_loc, 4, K], F32, name="vv")
            vf = persist.tile([128, b_loc, 4, K], F32, name="vf")
            pa_sb = persist.tile([1, b_loc, K], F32, name="pa_sb")
            dummy = persist.tile([1, 1], F32, name="dummy")

            stats_in = dram.tile([1, 2 * KG], F32, name="stats_in")
            stats_out = dram.tile([1, 2 * KG], F32, name="stats_out")


            # ---- phase A: load + transpose + assignment + BN stats ----
            # Token permutation: xh[b][p, t] = x[b, 16*p + t] with
            # partition p holding 16 consecutive tokens of each batch
            # ("(b p t) d" order). Tokens are exchangeable within a batch
            # (BN stats, softmax, vlad, a_sum are all order-invariant);
            # this makes every partition's HBM read contiguous, so each
            # batch loads with ~128 DMA descriptors in one instruction.
            # DMA instructions carry a fixed ~2-3us serial launch cost in
            # the scheduler, so phase A uses as few as possible: 4 batch
            # loads + 4 batch XBAR transposes. ALL loads are emitted
            # before any transpose (a load emitted after a transpose picks
            # up a false WAR edge and serializes).
            # constants first (small, on the SWDGE/scalar queues) so the
            # transpose stream on the sync queue is never interrupted;
            # preload the Sqrt act table while the Act engine is idle
            nc.scalar.dma_start(ones_row[:], ones_row_c.ap()[:, :])
            nc.scalar.dma_start(gamma[:], gam[:, :])
            nc.scalar.dma_start(beta[:], bet[:, :])
            nc.gpsimd.dma_start(
                clh[:], cl.ap().rearrange("(c p) k -> p c k", p=128))
            nc.gpsimd.dma_start(
                c2n[:], c2.ap().rearrange("(c p) k -> p c k", p=128))
            nc.vector.memset(ones16[:], 1.0)
            nc.vector.memset(ones32[:], 1.0)
            nc.vector.memset(epsc[:], BN_EPS)
            nc.vector.memset(dummy[:], 1.0)
            nc.scalar.sqrt(dummy[:], dummy[:])
            xr = x.ap().rearrange("(b p t) d -> p b (t d)", p=128, t=TPB)
            for h in range(b_loc // 2):
                nc.gpsimd.dma_start(
                    xht[h][:].rearrange("p b t d -> p b (t d)"),
                    xr[:, 2 * h:2 * h + 2, :])
            with tc.tile_pool(name="psA", bufs=1, space="PSUM") as psA:
                pstat = psA.tile([1, 2, 2, KG], F32, name="pstat",
                                 tag="pstat", bufs=1)

                def emit_stats(q):
                    # accumulating [1, 2*2*KG=320] ones-matmuls per tile pair
                    for h in range(2):
                        s0 = 4 * q + 2 * h
                        nc.tensor.matmul(
                            pstat[:], ones16[:], aq[:, s0:s0 + 2, :, :],
                            start=(s0 == 0), stop=(s0 == nt - 2),
                            skip_group_check=True)

                # 8-tile (4096-column) slabs. On hardware the XBAR
                # transpose's completion semaphore fires when the ucode is
                # dispatched, not when the data lands, so a consumer keyed
                # on it races the transpose. All transposes go on ONE DGE
                # queue (FIFO): a transpose's dispatch therefore implies
                # the previous transpose's data is fully written. Each
                # slab's assignment matmuls are fenced by a guard matmul
                # that reads slot 0 of the NEXT slab's transpose; the last
                # slab is fenced by a small same-queue self-copy instead.
                slabs = [(t0s // TPB, t0s, 8) for t0s in range(0, nt, 8)]
                xhTgs = []

                def emit_slab(sj):
                    bj, t0j, nj = slabs[sj]
                    xhT = xhTgs[sj]
                    for q in range(nj // 4):
                        p1 = psA.tile([128, 4, KG], F32, name="p1",
                                      tag="p1", bufs=3)
                        for j in range(4):
                            for c in range(4):
                                nc.tensor.matmul(
                                    p1[:, j, :],
                                    xhT[:, 16 * q + 4 * j + c, :],
                                    clh[:, c, :], start=(c == 0),
                                    stop=(c == 3), skip_group_check=True)
                        # stats lag a chunk behind so the PE never waits
                        # on the DVE square of the current chunk
                        qq = t0j // 4 + q
                        if qq >= 1:
                            emit_stats(qq - 1)
                        h0 = 4 * qq
                        nc.vector.tensor_copy(aq[:, h0:h0 + 4, 0, :],
                                              p1[:])
                        with nc.allow_low_precision("fp16 logit squares"):
                            nc.vector.tensor_tensor(
                                aq[:, h0:h0 + 4, 1, :],
                                aq[:, h0:h0 + 4, 0, :],
                                aq[:, h0:h0 + 4, 0, :], op=OP.mult)

                for si, (b, t0s, ntl) in enumerate(slabs):
                    xhTg = work.tile([128, 32, 128], F16, name="xhTg",
                                     tag="xhT", bufs=4)
                    # XBAR transpose: (128, ntl*512) -> (128, 4*ntl, 128)
                    # with logical row 128*e + p at [:, e, :]; e = 4*j + c,
                    # d = 128*c + p (chunk-major per tile), matching clh
                    toff = t0s - TPB * b
                    nc.sync.dma_start(
                        xhTg[:, :4 * ntl, :],
                        xht[b // 2][:, b % 2, toff:toff + ntl, :],
                        transpose=True)
                    xhTgs.append(xhTg)
                    if si >= 1:
                        pguard = psA.tile([128, 1], F32, name="pguard",
                                          tag="guard", bufs=2)
                        nc.tensor.matmul(pguard[:], xhTg[:, 0, :],
                                         ones16[:], start=True, stop=True,
                                         skip_group_check=True)
                        emit_slab(si - 1)
                # last slab: same-queue self-copy guard (the data dep
                # on slot 31 pins it after the transpose in queue order)
                nc.sync.dma_start(xhTgs[-1][:, 31, :], xhTgs[-1][:, 31, :])
                pguard = psA.tile([128, 1], F32, name="pguard",
                                  tag="guard", bufs=2)
                nc.tensor.matmul(pguard[:], xhTgs[-1][:, 31, :], ones16[:],
                                 start=True, stop=True,
                                 skip_group_check=True)
                emit_slab(len(slabs) - 1)
                emit_stats(nt // 4 - 1)

                # fold the tile-pair axis: [1,(i,s,k)] -> [1,(s,k)]
                nc.vector.tensor_reduce(
                    stats_sb[:].rearrange("p (s k) -> p s k", s=2),
                    pstat[:].rearrange("p i s k -> p s k i"),
                    axis=AX.X, op=OP.add)

            # ---- all-reduce stats ----
            nc.sync.dma_start(stats_in[:], stats_sb[:])
            if with_collective:
                nc.gpsimd.collective_compute(
                    "AllReduce", OP.add,
                    replica_groups=[list(range(n_cores))],
                    ins=[stats_in.opt()], outs=[stats_out.opt()])
            else:
                nc.sync.dma_start(stats_out[:], stats_in[:])
            nc.sync.dma_start(stats_g[:], stats_out[:])

            # ---- BN affine: scale = gamma*rsqrt(var+eps); shift = beta-mean*scale
            mq = work.tile([1, 2 * KG], F32, name="mq", tag="sv", bufs=6)
            msq = work.tile([1, KG], F32, name="msq", tag="sv", bufs=6)
            var = work.tile([1, KG], F32, name="var", tag="sv", bufs=6)
            sd = work.tile([1, KG], F32, name="sd", tag="sv", bufs=6)
            rsd = work.tile([1, KG], F32, name="rsd", tag="sv", bufs=6)
            t1 = work.tile([1, KG], F32, name="t1", tag="sv", bufs=6)
            inv_n = 1.0 / float(total_tok)
            nc.vector.tensor_scalar_mul(mq[:], stats_g[:], inv_n)
            nc.vector.tensor_tensor(msq[:], mq[:, :KG], mq[:, :KG], op=OP.mult)
            nc.vector.tensor_tensor(var[:], mq[:, KG:], msq[:], op=OP.subtract)
            nc.scalar.activation(sd[:], var[:], ACTF.Sqrt, bias=epsc[:])
            nc.vector.reciprocal(rsd[:], sd[:])
            nc.vector.tensor_tensor(ss[:, :KG], rsd[:], gamma[:], op=OP.mult)
            nc.vector.tensor_tensor(t1[:], mq[:, :KG], ss[:, :KG], op=OP.mult)
            nc.vector.tensor_tensor(ss[:, KG:], beta[:], t1[:], op=OP.subtract)

            # ---- phase B: softmax + vlad + normalization ----
            with tc.tile_pool(name="psB", bufs=1, space="PSUM") as psB:
                pbc = psB.tile([128, 2 * KG], F32, name="pbc",
                               tag="misc", bufs=2)
                nc.tensor.matmul(pbc[:], ones_row[:], ss[:], start=True,
                                 stop=True, skip_group_check=True)
                nc.vector.tensor_copy(bcB[:].rearrange("p s k -> p (s k)"),
                                      pbc[:])
                scale_b = bcB[:, 0:1, :]
                shift_b = bcB[:, 1:2, :]

                te_tiles = {}

                def s1a(b):
                    # te = assn*scale + shift; exp. For b0 the ops are
                    # emitted in two half-batch pieces so the first
                    # exp/denominator can start ~1.5us earlier (pipeline
                    # fill); later batches overlap and use one piece.
                    t0 = b * TPB
                    te = work.tile([128, TPB, KG], F16, name="te",
                                   tag="te", bufs=2)
                    pieces = ((0, TPB // 2), (TPB // 2, TPB)) if b == 0 \
                        else ((0, TPB),)
                    for (ta, tb) in pieces:
                        n = tb - ta
                        nc.vector.tensor_tensor(
                            te[:, ta:tb], aq[:, t0 + ta:t0 + tb, 0, :],
                            scale_b.to_broadcast([128, n, KG]), op=OP.mult)
                        nc.vector.tensor_tensor(
                            te[:, ta:tb], te[:, ta:tb],
                            shift_b.to_broadcast([128, n, KG]), op=OP.add)
                        nc.scalar.activation(te[:, ta:tb], te[:, ta:tb],
                                             ACTF.Exp)
                    te_tiles[b] = te

                def s1b(b):
                    # denominators + renormalize; renorm split DVE/Act
                    # (DVE produces the first half, which the vlad matmul
                    # consumes first). b0 is emitted per half so its first
                    # sm tiles are ready ~1.5us sooner (pipeline fill).
                    t0 = b * TPB
                    te = te_tiles.pop(b)
                    half = TPB // 2
                    denom = work.tile([128, TPB], F16, name="denom",
                                      tag="dn", bufs=2)
                    # fp32: the Act Copy's scale AP must be FP32
                    recip = work.tile([128, TPB], F32, name="recip",
                                      tag="rc", bufs=2)
                    pieces = ((0, half), (half, TPB)) if b == 0 \
                        else ((0, TPB),)
                    for (ta, tb) in pieces:
                        with nc.allow_low_precision("fp16 softmax denom"):
                            nc.vector.tensor_reduce(
                                denom[:, ta:tb], te[:, ta:tb], axis=AX.X,
                                op=OP.add)
                        nc.vector.reciprocal(recip[:, ta:tb],
                                              denom[:, ta:tb])
                        mid = min(tb, max(ta, half))
                        if mid > ta:
                            nc.vector.tensor_tensor(
                                sm[:, t0 + ta:t0 + mid, :],
                                te[:, ta:mid, :K],
                                recip[:, ta:mid]
                                .rearrange("p (t a) -> p t a", a=1)
                                .to_broadcast([128, mid - ta, K]),
                                op=OP.mult)
                        if tb > mid:
                            with nc.allow_low_precision("fp16 softmax"):
                                nc.gpsimd.tensor_tensor(
                                    sm[:, t0 + mid:t0 + tb, :],
                                    te[:, mid:tb, :K],
                                    recip[:, mid:tb]
                                    .rearrange("p (t a) -> p t a", a=1)
                                    .to_broadcast([128, tb - mid, K]),
                                    op=OP.mult)

                def s2(b):
                    # vlad matmul with x stationary; the a_sum ones-matmuls
                    # go after the first chunk pass (all sm tiles consumed
                    # by then) so s3's pam/av overlap the remaining chunks
                    t0 = b * TPB
                    pas = psB.tile([1, K], F32, name="pas",
                                   tag="pas", bufs=2)
                    pv = psB.tile([128, 4, K], F32, name="pv",
                                  tag="pv", bufs=3)
                    # NOTE: groups must be contiguous per PSUM bank region --
                    # start=True clears has_written for the whole bank
                    for c in range(4):
                        for i in range(TPB):
                            t = t0 + i
                            nc.tensor.matmul(
                                pv[:, c, :],
                                xht[b // 2][:, b % 2, i,
                                            c * 128:(c + 1) * 128],
                                sm[:, t, :],
                                start=(i == 0), stop=(i == TPB - 1),
                                skip_group_check=True)
                        if c == 0:
                            for u in range(TPB):
                                nc.tensor.matmul(
                                    pas[:], ones16[:], sm[:, t0 + u, :],
                                    start=(u == 0), stop=(u == TPB - 1),
                                    skip_group_check=True)
                    return pv, pas

                def s3(b, pv, pas):
                    # pa_sb copy emitted here (one batch late) so the DVE
                    # queue never blocks on the PE's pas accumulation
                    nc.vector.tensor_copy(pa_sb[:, b, :], pas[:])
                    pam = psB.tile([128, K], F32, name="pam",
                                   tag="pam", bufs=1)
                    nc.tensor.matmul(pam[:], ones_row[:], pa_sb[:, b, :],
                                     start=True, stop=True,
                                     skip_group_check=True)
                    # GPSIMD cannot read PSUM on hardware: copy the pam
                    # broadcast to SBUF (DVE), compute av on GPSIMD, and do
                    # the pv subtraction on DVE
                    pam_sb = work.tile([128, K], F32, name="pam_sb",
                                       tag="pams", bufs=2)
                    nc.vector.tensor_copy(pam_sb[:], pam[:])
                    av = work.tile([128, 4, K], F32, name="av",
                                   tag="av", bufs=2)
                    nc.gpsimd.tensor_tensor(
                        av[:], c2n[:],
                        pam_sb[:].rearrange("p (a k) -> p a k", a=1)
                        .to_broadcast([128, 4, K]), op=OP.mult)
                    nc.vector.tensor_tensor(vv[:, b], pv[:], av[:],
                                            op=OP.subtract)

                yb = y.ap().rearrange("b (c p k) -> p b c k", p=128, k=K)

                def tail_one(b):
                    # normalization for batch b: intra-norm 1/||v||; the
                    # global norm of the flattened intra-normalized vlad is
                    # exactly sqrt(K)=8 -> fold 1/8. n2 comes from an fp32
                    # matmul of vv against itself per chunk (diag not
                    # needed: lhsT=vv chunk, rhs=vv chunk gives k x k; too
                    # big) -- instead square on the PE is not possible, so
                    # n2 = ones^T (vv*vv) still needs the elementwise
                    # square; to keep it off the DVE it runs as an fp32
                    # matmul with vv as BOTH stationary and moving is
                    # invalid, so: square on GPSIMD (idle here), fp32
                    # ones-matmul reduce, no DVE reduce.
                    sq = work.tile([128, 4, K], F32, name="sq",
                                   tag="sq", bufs=2)
                    nc.gpsimd.tensor_tensor(sq[:], vv[:, b], vv[:, b],
                                            op=OP.mult)
                    pnrm = psB.tile([1, 4, K], F32, name="pnrm",
                                    tag="misc", bufs=2)
                    nc.tensor.matmul(pnrm[:], ones32[:], sq[:], start=True,
                                     stop=True, skip_group_check=True)
                    n2 = work.tile([1, K], F32, name="n2", tag="n2",
                                   bufs=6)
                    nc.vector.tensor_reduce(
                        n2[:], pnrm[:].rearrange("p c k -> p k c"),
                        axis=AX.X, op=OP.add)
                    snorm = work.tile([1, K], F32, name="snorm",
                                      tag="n2", bufs=6)
                    nc.scalar.activation(snorm[:], n2[:], ACTF.Sqrt)
                    nc.vector.tensor_scalar(snorm[:], snorm[:], L2_EPS, 8.0,
                                            op0=OP.max, op1=OP.mult)
                    rn = work.tile([1, K], F32, name="rn", tag="n2",
                                   bufs=6)
                    nc.vector.reciprocal(rn[:], snorm[:])
                    prnB = psB.tile([128, K], F32, name="prnB",
                                    tag="misc", bufs=2)
                    nc.tensor.matmul(prnB[:], ones_row[:], rn[:],
                                     start=True, stop=True,
                                     skip_group_check=True)
                    nc.vector.tensor_tensor(
                        vf[:, b], vv[:, b],
                        prnB[:].rearrange("p (a k) -> p a k", a=1)
                        .to_broadcast([128, 4, K]), op=OP.mult)
                    dma_eng = nc.sync if b % 2 == 0 else nc.scalar
                    dma_eng.dma_start(yb[:, b], vf[:, b])

                # software pipeline: s1a one batch ahead; s3 lags one batch
                # so the PE's pam matmul never waits on the DVE reduce;
                # the first tail half (b0,b1) overlaps the b3 vlad work
                s1a(0)
                pvs = {}
                for b in range(b_loc):
                    if b + 1 < b_loc:
                        s1a(b + 1)
                    s1b(b)
                    pvs[b] = s2(b)
                    if b >= 1:
                        s3(b - 1, *pvs.pop(b - 1))
                        tail_one(b - 1)
                s3(b_loc - 1, *pvs.pop(b_loc - 1))
                tail_one(b_loc - 1)
                if debug_taps:
                    nc.gpsimd.dma_start(
                        dbg_aq.ap().rearrange("p (t s k) -> p t s k",
                                              t=nt, s=2), aq[:])
                    nc.gpsimd.dma_start(
                        dbg_sm.ap().rearrange("p (t k) -> p t k", t=nt),
                        sm[:])
                    nc.sync.dma_start(dbg_st.ap()[:, :], stats_g[:])
                    nc.sync.dma_start(
                        dbg_vv.ap().rearrange("p (b c k) -> p b c k",
                                              b=b_loc, c=4), vv[:])
                    nc.gpsimd.dma_start(
                        dbg_xh.ap().rearrange("p (b t d) -> p b t d",
                                              b=2, t=TPB), xht[0][:])
                    nc.gpsimd.dma_start(
                        dbg_cl.ap().rearrange("p (c k) -> p c k", c=4),
                        clh[:])
    nc.compile()
    return nc


_CACHE = {}


def _get(b_loc, n_cores, with_collective):
    key = (b_loc, n_cores, with_collective)
    if key not in _CACHE:
        _CACHE[key] = build(b_loc, n_cores, with_collective)
    return _CACHE[key]


def make_in_maps(x, clusters, clusters2, bn_gamma, bn_beta, n_cores=N_CORES):
    B = x.shape[0]
    b_loc = B // n_cores
    shared = {
        "clusters": np.ascontiguousarray(clusters, np.float32),
        "clusters2": np.ascontiguousarray(
            np.asarray(clusters2).reshape(D, K), np.float32),
        "bn_gamma": np.ascontiguousarray(
            np.asarray(bn_gamma).reshape(1, KG), np.float32),
        "bn_beta": np.ascontiguousarray(
            np.asarray(bn_beta).reshape(1, KG), np.float32),
    }
    in_maps = []
    for i in range(n_cores):
        m = dict(shared)
        m["x"] = np.ascontiguousarray(
            np.asarray(x[i * b_loc:(i + 1) * b_loc]).reshape(
                b_loc * N_SEQ, D), np.float32)
        in_maps.append(m)
    return in_maps


def kernel(x, clusters, clusters2, bn_gamma, bn_beta):
    B, N, Dd = x.shape
    assert (N, Dd) == (N_SEQ, D) and B % N_CORES == 0
    b_loc = B // N_CORES
    nc = _get(b_loc, N_CORES, True)
    in_maps = make_in_maps(x, clusters, clusters2, bn_gamma, bn_beta)
    res = run_bass_kernel_spmd(nc, in_maps, core_ids=list(range(N_CORES)))
    out = np.concatenate([res.results[i]["y"] for i in range(N_CORES)], axis=0)
    return out
